# revision 17
# baseline (speedup 1.0000x reference)
"""Trainium2 Bass kernel for nn_DiDA_80358838108451.

Pipeline (reference semantics):
  feats_q  = img_q  @ W_enc                [B,F]
  logits_q = feats_q @ W_fc + b_fc         [B,C]   (same for img_q1)
  labels   = argmax(softmax(logits_q)*partial_Y)   [B]
  new_proto = sequential per-class EMA of feats_q (closed form), normalized
  prot_scores = new_proto @ W_fc + b_fc    [C,C]

Sharding: data-parallel over batch, 128 rows per core on 8 cores; weights
replicated (W_enc kept fully resident in SBUF). The order-dependent EMA uses
the closed form
  new_proto[c] = w^total[c] * proto[c] + sum_i (1-w) w^{c_after(i)} feats_i
with c_after(i) = #(later same-label samples in global batch order). Per
core, within-shard occurrence counts come from a triangular matmul; per-class
shard counts are AllGathered (8x384 floats) to form global suffix counts;
then both the locally scaled scatter S_k = onehot^T @ (coef*feats) and its
projection P_k = S_k @ W_fc ride one ReduceScatter, so core k finishes the
prototype update and prototype scores for its own 48-class chunk with a few
row-scaled vector ops:  pscores = rinv*(w^tot * proto@W_fc + P) + b.

Matmul layouts: the contraction dim must be on partitions for both operands,
so the host passes img shards pre-transposed [D,128]. Scatter/segment-sum is
a matmul with the one-hot matrix as the stationary operand. Classes are
padded 345->384 so collective chunks are 32B-aligned and divisible by 8.

The two encoder matmuls run as float32r (fp22 reduced precision, ~2x the
fp32 rate on hardware). The label argmax uses an exact masking trick
(candidates keep their logits bit-exactly), and the fp22 logit error
(~2e-4 rel) is far below the minimum top-2 candidate margin (4e-4 absolute,
typically ~0.25), so labels match the fp32 reference exactly (verified).
Logits / prototype-score matmuls stay fp32.

DMA plan: sync queue streams W_enc then imgT_q1; scalar queue streams
imgT_q; gpsimd carries constants, outputs and collective bounce buffers.
"""

import os

import numpy as np

import concourse.bacc as bacc
import concourse.tile as tile
import concourse.mybir as mybir
from concourse import bass_utils

B, D, C, F = 1024, 8192, 345, 512
PROTO_W = 0.99
NCORES = 8
SH = B // NCORES            # 128 rows per core
KT = D // 128               # 64 k-tiles for the encoder matmul
FJ = F // 128               # 4 f-chunks
CPAD = 384                  # classes padded for alignment / 8-way split
COWN = CPAD // NCORES       # 48 classes finished per core
NCH = CPAD // 128           # 3 class chunks of 128
WCH = 8                     # W_enc k-tiles per DMA (2 MB)
ICH = 4                     # img k-tiles per DMA (256 KB)
CW = 352                    # W_fc columns padded with zeros
RSW = F + CW                # ReduceScatter row width: S | P(padded)
BIG = 1024.0                # > CPAD, for first-argmax trick

F32 = mybir.dt.float32
F32R = mybir.dt.float32r
# float32r: encoder matmuls in fp22 (fast). float32: full precision.
ENC_DT = F32R if os.environ.get("ENC_DT", "f32r") == "f32r" else F32

_CACHE = {}


def _build():
    nc = bacc.Bacc("TRN2", target_bir_lowering=False, debug=False,
                   num_devices=NCORES)
    dt = mybir.dt

    # ---- I/O ----
    imgT_q = nc.dram_tensor("imgT_q", [D, SH], ENC_DT, kind="ExternalInput")
    imgT_q1 = nc.dram_tensor("imgT_q1", [D, SH], ENC_DT, kind="ExternalInput")
    wenc = nc.dram_tensor("wenc", [D, F], ENC_DT, kind="ExternalInput")
    py = nc.dram_tensor("py", [SH, C], F32, kind="ExternalInput")
    wfc = nc.dram_tensor("wfc", [F, C], F32, kind="ExternalInput")
    proto_own = nc.dram_tensor("proto_own", [COWN, F], F32, kind="ExternalInput")
    sel3 = nc.dram_tensor("sel3", [128, NCH, COWN], F32, kind="ExternalInput")
    biasb = nc.dram_tensor("biasb", [SH, C], F32, kind="ExternalInput")
    idx = nc.dram_tensor("idx", [SH, CPAD], F32, kind="ExternalInput")
    ident = nc.dram_tensor("ident", [128, 128], F32, kind="ExternalInput")
    triT = nc.dram_tensor("triT", [128, 128], F32, kind="ExternalInput")
    suffmask = nc.dram_tensor("suffmask", [NCORES, 1], F32, kind="ExternalInput")

    feats_q_o = nc.dram_tensor("feats_q", [SH, F], F32, kind="ExternalOutput")
    feats_q1_o = nc.dram_tensor("feats_q1", [SH, F], F32, kind="ExternalOutput")
    logits_q_o = nc.dram_tensor("logits_q", [SH, C], F32, kind="ExternalOutput")
    logits_q1_o = nc.dram_tensor("logits_q1", [SH, C], F32, kind="ExternalOutput")
    labels_o = nc.dram_tensor("labels", [SH, 1], dt.int32, kind="ExternalOutput")
    nproto_own_o = nc.dram_tensor("nproto_own", [COWN, F], F32,
                                  kind="ExternalOutput")
    pscores_own_o = nc.dram_tensor("pscores_own", [COWN, C], F32,
                                   kind="ExternalOutput")

    Act = mybir.ActivationFunctionType
    Alu = mybir.AluOpType
    Ax = mybir.AxisListType
    lnw = float(np.log(PROTO_W))
    ln1mw = float(np.log(1.0 - PROTO_W))
    groups = [list(range(NCORES))]

    with tile.TileContext(nc) as tc:
        with (
            tc.tile_pool(name="cst", bufs=1) as cst,
            tc.tile_pool(name="wres", bufs=1) as wres,
            tc.tile_pool(name="img", bufs=2) as imgp,
            tc.tile_pool(name="big", bufs=1) as bigp,
            tc.tile_pool(name="wk", bufs=1) as wk,
            tc.tile_pool(name="psf", bufs=2, space="PSUM") as psf,
            tc.tile_pool(name="pst", bufs=2, space="PSUM") as pstp,
            tc.tile_pool(name="psm", bufs=3, space="PSUM") as psm,
            tc.tile_pool(name="psr", bufs=1, space="PSUM") as psr,
            tc.tile_pool(name="dram", bufs=1, space="DRAM") as dram,
        ):
            _scrn = [0]

            def scr(p=128, w=512):
                _scrn[0] += 1
                t = wk.tile([128, 512], F32, tag="scr", bufs=4,
                            name=f"scr{_scrn[0]}")
                return t[0:p, 0:w]

            # ---- compute-critical streams first ----
            # sync queue: all of W_enc (resident), then imgT_q1
            wenc_sb = wres.tile([128, KT, F], ENC_DT)
            for c0 in range(0, KT, WCH):
                nc.sync.dma_start(
                    wenc_sb[:, c0:c0 + WCH, :],
                    wenc.rearrange("(a p) f -> p a f", p=128)[:, c0:c0 + WCH, :])
            # scalar queue: imgT_q
            iq_chunks = []
            for c0 in range(0, KT, ICH):
                itc = imgp.tile([128, ICH, SH], ENC_DT, tag="iq", name=f"iq{c0}")
                nc.scalar.dma_start(
                    itc[:],
                    imgT_q.rearrange("(a p) b -> p a b", p=128)[:, c0:c0 + ICH, :])
                iq_chunks.append(itc)

            # gpsimd: constants
            wfc_sb = cst.tile([128, FJ, CW], F32)
            nc.gpsimd.memset(wfc_sb[:, :, C:CW], 0.0)
            nc.gpsimd.dma_start(wfc_sb[:, :, 0:C],
                                wfc.rearrange("(a p) c -> p a c", p=128))
            py_sb = cst.tile([SH, C], F32)
            nc.gpsimd.dma_start(py_sb[:], py[:])
            idx_sb = cst.tile([SH, CPAD], F32)
            nc.gpsimd.dma_start(idx_sb[:], idx[:])
            biasb_sb = cst.tile([SH, C], F32)
            nc.gpsimd.dma_start(biasb_sb[:], biasb[:])
            ident_sb = cst.tile([128, 128], F32)
            nc.gpsimd.dma_start(ident_sb[:], ident[:])
            triT_sb = cst.tile([128, 128], F32)
            nc.gpsimd.dma_start(triT_sb[:], triT[:])
            suff_sb = cst.tile([NCORES, 1], F32)
            nc.gpsimd.dma_start(suff_sb[:], suffmask[:])
            proto_own_sb = cst.tile([COWN, F], F32)
            nc.gpsimd.dma_start(proto_own_sb[:], proto_own[:])
            sel3_sb = cst.tile([128, NCH, COWN], F32)
            nc.gpsimd.dma_start(sel3_sb[:], sel3[:])
            ones128 = cst.tile([128, 1], F32)
            nc.gpsimd.memset(ones128[:], 1.0)
            ones8 = cst.tile([NCORES, 1], F32)
            nc.gpsimd.memset(ones8[:], 1.0)
            ones1r = cst.tile([1, 128], F32)
            nc.gpsimd.memset(ones1r[:], 1.0)
            ln1mw_t = cst.tile([SH, 1], F32)
            nc.gpsimd.memset(ln1mw_t[:], ln1mw)

            # ---- phase 1: feats_q ----
            psum_fq = psf.tile([128, F], F32, tag="f")
            for kt in range(KT):
                nc.tensor.matmul(psum_fq[:], iq_chunks[kt // ICH][:, kt % ICH, :],
                                 wenc_sb[:, kt, :],
                                 start=(kt == 0), stop=(kt == KT - 1))
            fq_sb = bigp.tile([SH, F], F32)
            nc.vector.tensor_copy(fq_sb[:], psum_fq[:])
            nc.gpsimd.dma_start(feats_q_o[:], fq_sb[:])

            # ---- phase 2: logits_q ----
            ftq = bigp.tile([128, FJ, SH], F32)
            for j in range(FJ):
                ptr = pstp.tile([128, 128], F32, tag="tr", name=f"trq{j}")
                nc.tensor.transpose(ptr[:], fq_sb[:, j * 128:(j + 1) * 128],
                                    ident_sb[:])
                nc.vector.tensor_copy(ftq[:, j, :], ptr[:])
            psum_lq = psm.tile([SH, C], F32, tag="med", name="psum_lq")
            for j in range(FJ):
                nc.tensor.matmul(psum_lq[:], ftq[:, j, :], wfc_sb[:, j, 0:C],
                                 start=(j == 0), stop=(j == FJ - 1))
            logits_q_sb = wk.tile([SH, C], F32)
            nc.vector.tensor_tensor(logits_q_sb[:], psum_lq[:], biasb_sb[:], Alu.add)
            nc.gpsimd.dma_start(logits_q_o[:], logits_q_sb[:])

            # ---- phase 3: labels / onehot / local counts -> AllGather ----
            neg = scr(SH, C)
            nc.vector.tensor_scalar(neg, py_sb[:], 1e9, -1e9, Alu.mult, Alu.add)
            masked = scr(SH, C)
            nc.vector.tensor_tensor(masked, logits_q_sb[:], py_sb[:], Alu.mult)
            nc.vector.tensor_tensor(masked, masked, neg, Alu.add)
            mx = wk.tile([SH, 1], F32)
            nc.vector.tensor_reduce(mx[:], masked, axis=Ax.X, op=Alu.max)
            eq = scr(SH, C)
            nc.vector.tensor_scalar(eq, masked, mx[:], None, Alu.is_equal)
            bmi = scr(SH, C)
            nc.vector.tensor_scalar(bmi, idx_sb[:, 0:C], -1.0, BIG,
                                    Alu.mult, Alu.add)
            fs = scr(SH, C)
            nc.vector.tensor_tensor(fs, eq, bmi, Alu.mult)
            rmx = wk.tile([SH, 1], F32)
            nc.vector.tensor_reduce(rmx[:], fs, axis=Ax.X, op=Alu.max)
            labf = wk.tile([SH, 1], F32)
            nc.vector.tensor_scalar(labf[:], rmx[:], -1.0, BIG, Alu.mult, Alu.add)
            lab_i32 = wk.tile([SH, 1], mybir.dt.int32)
            nc.vector.tensor_copy(lab_i32[:], labf[:])
            nc.gpsimd.dma_start(labels_o[:], lab_i32[:])
            onehot = bigp.tile([SH, CPAD], F32)
            nc.vector.tensor_scalar(onehot[:], idx_sb[:], labf[:], None,
                                    Alu.is_equal)

            psum_cnt = psr.tile([1, CPAD], F32, tag="row", name="psum_cnt")
            nc.tensor.matmul(psum_cnt[:], ones128[:], onehot[:],
                             start=True, stop=True)
            cnt_sb = wk.tile([1, CPAD], F32)
            nc.vector.tensor_copy(cnt_sb[:], psum_cnt[:])
            ag_in = dram.tile([1, CPAD], F32)
            ag_out = dram.tile([NCORES, CPAD], F32, addr_space="Shared")
            nc.gpsimd.dma_start(ag_in[:], cnt_sb[:])
            nc.gpsimd.collective_compute(
                "AllGather", Alu.bypass, replica_groups=groups,
                ins=[ag_in.opt()], outs=[ag_out.opt()])
            cnts8 = bigp.tile([NCORES, CPAD], F32)
            nc.gpsimd.dma_start(cnts8[:], ag_out[:])

            # occ = (# earlier same-label samples in shard)
            psum_b = psm.tile([SH, CPAD], F32, tag="med", name="psum_b")
            nc.tensor.matmul(psum_b[:], triT_sb[:], onehot[:],
                             start=True, stop=True)
            tmp_b = scr(SH, CPAD)
            nc.vector.tensor_tensor(tmp_b, psum_b[:], onehot[:], Alu.mult)
            occ = wk.tile([SH, 1], F32)
            nc.vector.tensor_reduce(occ[:], tmp_b, axis=Ax.X, op=Alu.add)

            # ---- filler (no AG dependency): proto_own @ W_fc ----
            ppT = bigp.tile([128, FJ, COWN], F32)
            for j in range(FJ):
                ptp = pstp.tile([128, 128], F32, tag="tr", name=f"trpp{j}")
                nc.tensor.transpose(ptp[:, 0:COWN],
                                    proto_own_sb[:, j * 128:(j + 1) * 128],
                                    ident_sb[0:COWN, 0:COWN])
                nc.vector.tensor_copy(ppT[:, j, :], ptp[:, 0:COWN])
            psum_pp = psm.tile([COWN, C], F32, tag="med", name="psum_pp")
            for j in range(FJ):
                nc.tensor.matmul(psum_pp[:], ppT[:, j, :], wfc_sb[:, j, 0:C],
                                 start=(j == 0), stop=(j == FJ - 1))
            PP_sb = wk.tile([COWN, C], F32)
            nc.vector.tensor_copy(PP_sb[:], psum_pp[:])

            # ---- phase 5: feats_q1 (imgT_q1 streams on sync after wenc) ----
            psum_fq1 = psf.tile([128, F], F32, tag="f")
            iq1_chunks = []
            for c0 in range(0, KT, ICH):
                itc1 = imgp.tile([128, ICH, SH], ENC_DT, tag="iq1",
                                 name=f"iq1_{c0}")
                nc.sync.dma_start(
                    itc1[:],
                    imgT_q1.rearrange("(a p) b -> p a b", p=128)[:, c0:c0 + ICH, :])
                iq1_chunks.append(itc1)
            for kt in range(KT):
                nc.tensor.matmul(psum_fq1[:], iq1_chunks[kt // ICH][:, kt % ICH, :],
                                 wenc_sb[:, kt, :],
                                 start=(kt == 0), stop=(kt == KT - 1))

            # ---- phase 4: suffix counts, coef, scatter, ReduceScatter ----
            psum_A = psr.tile([1, CPAD], F32, tag="row", name="psum_A")
            nc.tensor.matmul(psum_A[:], suff_sb[:], cnts8[:], start=True, stop=True)
            A_sb = wk.tile([1, CPAD], F32)
            nc.vector.tensor_copy(A_sb[:], psum_A[:])
            psum_Ab = psm.tile([SH, CPAD], F32, tag="med", name="psum_Ab")
            nc.tensor.matmul(psum_Ab[:], ones1r[:], A_sb[:],
                             start=True, stop=True)
            tmp_g = scr(SH, CPAD)
            nc.vector.tensor_tensor(tmp_g, psum_Ab[:], onehot[:], Alu.mult)
            gsum = wk.tile([SH, 1], F32)
            nc.vector.tensor_reduce(gsum[:], tmp_g, axis=Ax.X, op=Alu.add)
            c_after = wk.tile([SH, 1], F32)
            nc.vector.tensor_scalar(c_after[:], gsum[:], -1.0, None, Alu.add)
            nc.vector.tensor_tensor(c_after[:], c_after[:], occ[:], Alu.subtract)
            coef = wk.tile([SH, 1], F32)
            nc.scalar.activation(coef[:], c_after[:], Act.Exp, bias=ln1mw_t[:],
                                 scale=lnw)
            cf = bigp.tile([SH, F], F32)
            nc.vector.tensor_scalar(cf[:], fq_sb[:], coef[:], None, Alu.mult)

            rs_in = dram.tile([CPAD, RSW], F32)
            rs_out = dram.tile([COWN, RSW], F32)
            # S^T chunks (for P = S @ W_fc)
            skT = bigp.tile([128, FJ, CPAD], F32)
            for j in range(FJ):
                pskt = psm.tile([SH, CPAD], F32, tag="med", name=f"pskt{j}")
                nc.tensor.matmul(pskt[:], cf[:, j * 128:(j + 1) * 128], onehot[:],
                                 start=True, stop=True)
                nc.vector.tensor_copy(skT[:, j, :], pskt[:])
            # S chunks -> rs_in[:, 0:F]
            for ch in range(NCH):
                pS = psm.tile([128, F], F32, tag="med", name=f"pS{ch}")
                nc.tensor.matmul(pS[:], onehot[:, ch * 128:(ch + 1) * 128], cf[:],
                                 start=True, stop=True)
                s_sb = scr()
                nc.vector.tensor_copy(s_sb, pS[:])
                nc.gpsimd.dma_start(rs_in[ch * 128:(ch + 1) * 128, 0:F], s_sb)
            # P chunks -> rs_in[:, F:F+C]
            for ch in range(NCH):
                pP = psm.tile([128, CW], F32, tag="med", name=f"pP{ch}")
                for j in range(FJ):
                    nc.tensor.matmul(pP[:], skT[:, j, ch * 128:(ch + 1) * 128],
                                     wfc_sb[:, j, :],
                                     start=(j == 0), stop=(j == FJ - 1))
                p_sb = scr(128, CW)
                nc.vector.tensor_copy(p_sb, pP[:])
                nc.gpsimd.dma_start(rs_in[ch * 128:(ch + 1) * 128, F:F + CW], p_sb)
            nc.gpsimd.collective_compute(
                "ReduceScatter", Alu.add, replica_groups=groups,
                ins=[rs_in.opt()], outs=[rs_out.opt()])
            S_own = bigp.tile([COWN, F], F32)
            nc.gpsimd.dma_start(S_own[:], rs_out[:, 0:F])
            P_own = bigp.tile([COWN, C], F32)
            nc.gpsimd.dma_start(P_own[:], rs_out[:, F:F + C])

            # total counts for own classes: total_own = sel^T @ (cnts8^T @ 1)
            totp_sb = wk.tile([128, NCH, 1], F32)
            for ch in range(NCH):
                ptot = psr.tile([128, 1], F32, tag="row", name=f"ptot{ch}")
                nc.tensor.matmul(ptot[:], cnts8[:, ch * 128:(ch + 1) * 128],
                                 ones8[:], start=True, stop=True)
                nc.vector.tensor_copy(totp_sb[:, ch, :], ptot[:])
            ptot_own = psr.tile([COWN, 1], F32, tag="row", name="ptot_own")
            for ch in range(NCH):
                nc.tensor.matmul(ptot_own[:], sel3_sb[:, ch, :],
                                 totp_sb[:, ch, :],
                                 start=(ch == 0), stop=(ch == NCH - 1))
            wtot_own = wk.tile([COWN, 1], F32)
            nc.scalar.activation(wtot_own[:], ptot_own[:], Act.Exp, bias=0.0,
                                 scale=lnw)

            # ---- phase 5b: feats_q1 epilogue + logits_q1 ----
            fq1_sb = bigp.tile([SH, F], F32)
            nc.vector.tensor_copy(fq1_sb[:], psum_fq1[:])
            nc.gpsimd.dma_start(feats_q1_o[:], fq1_sb[:])
            for j in range(FJ):
                ptr1 = pstp.tile([128, 128], F32, tag="tr", name=f"trq1{j}")
                nc.tensor.transpose(ptr1[:], fq1_sb[:, j * 128:(j + 1) * 128],
                                    ident_sb[:])
                nc.vector.tensor_copy(ftq[:, j, :], ptr1[:])
            psum_lq1 = psm.tile([SH, C], F32, tag="med", name="psum_lq1")
            for j in range(FJ):
                nc.tensor.matmul(psum_lq1[:], ftq[:, j, :], wfc_sb[:, j, 0:C],
                                 start=(j == 0), stop=(j == FJ - 1))
            logits_q1_sb = wk.tile([SH, C], F32)
            nc.vector.tensor_tensor(logits_q1_sb[:], psum_lq1[:], biasb_sb[:],
                                    Alu.add)
            nc.gpsimd.dma_start(logits_q1_o[:], logits_q1_sb[:])

            # ---- phase 6: own-chunk proto update + scores ----
            newp = bigp.tile([COWN, F], F32)
            nc.vector.tensor_scalar(newp[:], proto_own_sb[:], wtot_own[:], None,
                                    Alu.mult)
            nc.vector.tensor_tensor(newp[:], newp[:], S_own[:], Alu.add)
            sqo = scr(COWN, F)
            ssq = wk.tile([COWN, 1], F32)
            nc.scalar.activation(sqo, newp[:], Act.Square, accum_out=ssq[:])
            snrm = wk.tile([COWN, 1], F32)
            nc.scalar.activation(snrm[:], ssq[:], Act.Sqrt)
            rinv = wk.tile([COWN, 1], F32)
            nc.vector.reciprocal(rinv[:], snrm[:])
            newpn = bigp.tile([COWN, F], F32)
            nc.vector.tensor_scalar(newpn[:], newp[:], rinv[:], None, Alu.mult)
            nc.gpsimd.dma_start(nproto_own_o[:], newpn[:])

            ps1 = scr(COWN, C)
            nc.vector.tensor_scalar(ps1, PP_sb[:], wtot_own[:], None, Alu.mult)
            nc.vector.tensor_tensor(ps1, ps1, P_own[:], Alu.add)
            nc.vector.tensor_scalar(ps1, ps1, rinv[:], None, Alu.mult)
            ps_sb = wk.tile([COWN, C], F32)
            nc.vector.tensor_tensor(ps_sb[:], ps1, biasb_sb[0:COWN, :], Alu.add)
            nc.gpsimd.dma_start(pscores_own_o[:], ps_sb[:])

    nc.compile()
    return nc


def _get_nc():
    if "nc" not in _CACHE:
        _CACHE["nc"] = _build()
    return _CACHE["nc"]


def _host_prep(img_q, img_q1, partial_Y, W_enc, W_fc, b_fc, proto):
    img_q = np.ascontiguousarray(img_q, dtype=np.float32)
    img_q1 = np.ascontiguousarray(img_q1, dtype=np.float32)
    partial_Y = np.ascontiguousarray(partial_Y, dtype=np.float32)
    W_enc = np.ascontiguousarray(W_enc, dtype=np.float32)
    W_fc = np.ascontiguousarray(W_fc, dtype=np.float32)
    b_fc = np.ascontiguousarray(b_fc, dtype=np.float32)
    proto = np.ascontiguousarray(proto, dtype=np.float32)

    # classes padded to CPAD; pad prototype rows are unit vectors so their
    # normalization is well-defined (host discards them)
    proto_pad = np.full((CPAD, F), 1.0 / np.sqrt(F), np.float32)
    proto_pad[:C] = proto
    biasb = np.ascontiguousarray(np.broadcast_to(b_fc[None, :], (SH, C)))
    idx = np.ascontiguousarray(
        np.broadcast_to(np.arange(CPAD, dtype=np.float32)[None, :], (SH, CPAD)))
    ident = np.eye(128, dtype=np.float32)
    triT = np.triu(np.ones((128, 128), dtype=np.float32), k=1)

    in_maps = []
    for k in range(NCORES):
        sl = slice(k * SH, (k + 1) * SH)
        suff = np.zeros((NCORES, 1), np.float32)
        suff[k:, 0] = 1.0
        sel = np.zeros((CPAD, COWN), np.float32)
        sel[k * COWN:(k + 1) * COWN] = np.eye(COWN, dtype=np.float32)
        in_maps.append({
            "imgT_q": np.ascontiguousarray(img_q[sl].T),
            "imgT_q1": np.ascontiguousarray(img_q1[sl].T),
            "py": np.ascontiguousarray(partial_Y[sl]),
            "wenc": W_enc,
            "wfc": W_fc,
            "proto_own": np.ascontiguousarray(
                proto_pad[k * COWN:(k + 1) * COWN]),
            "sel3": np.ascontiguousarray(
                sel.reshape(CPAD // 128, 128, COWN).transpose(1, 0, 2)),
            "biasb": biasb,
            "idx": idx,
            "ident": ident,
            "triT": triT,
            "suffmask": suff,
        })
    return in_maps


def _assemble(r):
    logits_q = np.concatenate([r[k]["logits_q"] for k in range(NCORES)], axis=0)
    logits_q1 = np.concatenate([r[k]["logits_q1"] for k in range(NCORES)], axis=0)
    feats_q = np.concatenate([r[k]["feats_q"] for k in range(NCORES)], axis=0)
    feats_q1 = np.concatenate([r[k]["feats_q1"] for k in range(NCORES)], axis=0)
    labels = np.concatenate(
        [r[k]["labels"].reshape(SH) for k in range(NCORES)], axis=0).astype(np.int32)
    new_proto = np.concatenate(
        [r[k]["nproto_own"] for k in range(NCORES)], axis=0)[:C]
    prot_scores = np.concatenate(
        [r[k]["pscores_own"] for k in range(NCORES)], axis=0)[:C]
    return (logits_q, logits_q1, new_proto, prot_scores, labels,
            feats_q, feats_q1)


def kernel(img_q, img_q1, partial_Y, W_enc, W_fc, b_fc, proto):
    in_maps = _host_prep(img_q, img_q1, partial_Y, W_enc, W_fc, b_fc, proto)
    nc = _get_nc()
    res = bass_utils.run_bass_kernel_spmd(nc, in_maps,
                                          core_ids=list(range(NCORES)))
    _CACHE["last_results"] = res
    return _assemble(res.results)
